# revision 12
# baseline (speedup 1.0000x reference)
"""Trainium2 Bass kernel for a batched GAT layer (BGATLayer).

Reference computation (per batch b of B=16, N=1024 nodes, F=512 features):
    h   = x @ W                                   # [N, F]
    s1  = h @ a1 ; s2 = h @ a2                    # [N]
    e   = leakyrelu(s1[:,None] + s2[None,:], 0.2) # [N, N]
    att = softmax(e, axis=1)                      # row softmax
    out = elu(att @ h + beta * h)                 # [N, F]

Sharding: batch B=16 split across 8 NeuronCores (2 batches/core, data
parallel); W/a/beta replicated.

v2 schedule (from v1 trace analysis at 124.6us):
  * v1 lost ~40us to head-of-line blocking: the C phase (z -> lrelu -> exp)
    paces at ACT speed while independent A/B matmuls of the next batch sat
    behind the z matmuls in the PE queue, idling the PE for 13us and
    letting the HAM clock-gate drop it to 1.2 GHz for 20us.  v2 interleaves
    emit order: C(0) tiles alternate with A(1)/B(1) tiles, C(1) with DE(0).
  * z operands swapped to zl=[ones; s2], zr=[s1; ones] so s1 lands via an
    engine copy (partition 0) and only s2 needs a DMA (partition 1 is
    unreachable by compute engines); saves one roundtrip DMA per batch.
  * epilogue fused in [128,1024] pairs with the exp/min identity
    elu(v) = max(-relu(1 - exp(v)), v)  (exp(min(v,0)) = min(exp(v),1)),
    two balance variants (ACT-heavy / DVE-heavy); one output DMA per pair.
  * x loaded as 4 [128,1024] pair-tiles per batch, W as one [128,2048]
    tile (fewer, larger DMAs; fewer Sync-queue issue slots).
  * dense PE warmup matmuls before/inside phase A hold the HAM activity
    window busy so the PE reaches 2.4 GHz by ~12us instead of 23us.
  * fp32-written constants (identity, ones, w12 via DVE reduce) are read
    directly through f32r bitcast views: f32r is bit-compatible with fp32
    (proven by the x DMA path, which moves raw fp32 bytes into tiles the
    PE reads as f32r).
"""

import sys

sys.path.insert(0, "/opt/trn_rl_repo")

from contextlib import ExitStack

import numpy as np

import concourse.bacc as bacc
import concourse.bass as bass
import concourse.mybir as mybir
from concourse.bass_utils import run_bass_kernel_spmd
from concourse.masks import make_identity
from concourse.tile import TileContext

P = 128
N_NODES = 1024
F = 512
B_TOTAL = 16
N_CORES = 8
B_PER_CORE = B_TOTAL // N_CORES
NK = F // P  # 4 contraction chunks for x @ W
NN = N_NODES // P  # 8 node chunks
ALPHA = 0.2

F32 = mybir.dt.float32
F32R = mybir.dt.float32r
AL = mybir.AluOpType
AF = mybir.ActivationFunctionType


def _r(ap):
    """float32r view of an fp32 AP (PE reduced-precision matmul mode)."""
    return ap.bitcast(F32R)


def build_nc(mm_fp32: bool = False, beta_val: float = 1.0) -> bass.Bass:
    cast = (lambda ap: ap) if mm_fp32 else _r

    nc = bacc.Bacc("TRN2")
    x_d = nc.dram_tensor("x", [B_PER_CORE, N_NODES, F], F32, kind="ExternalInput")
    w_d = nc.dram_tensor("W", [F, F], F32, kind="ExternalInput")
    a_d = nc.dram_tensor("a", [2 * F, 1], F32, kind="ExternalInput")
    beta_d = nc.dram_tensor("beta", [1], F32, kind="ExternalInput")
    out_d = nc.dram_tensor("out", [B_PER_CORE, N_NODES, F], F32, kind="ExternalOutput")
    # scratch for the reciprocal-rowsum row->column roundtrip
    r_d = nc.dram_tensor("r_scratch", [B_PER_CORE, N_NODES], F32)

    with TileContext(nc) as tc, ExitStack() as ctx:
        # ---------------- pools ----------------
        singles = ctx.enter_context(tc.tile_pool(name="singles", bufs=1))
        xin = ctx.enter_context(tc.tile_pool(name="xin", bufs=3))
        xtp = ctx.enter_context(tc.tile_pool(name="xtp", bufs=2))
        hpool = ctx.enter_context(tc.tile_pool(name="hpool", bufs=16))
        spool = ctx.enter_context(tc.tile_pool(name="spool", bufs=1))
        utp = ctx.enter_context(tc.tile_pool(name="utp", bufs=16))
        cpool = ctx.enter_context(tc.tile_pool(name="cpool", bufs=2))
        epool = ctx.enter_context(tc.tile_pool(name="epool", bufs=2))
        # PSUM (8 banks): zps 2x[128,1024] = 4, hps 2x[128,512] = 1? no: 2,
        # sps 1x[2,1024] (allocates full partition width) = 2.
        psZ = ctx.enter_context(tc.tile_pool(name="psZ", bufs=2, space="PSUM"))
        psH = ctx.enter_context(tc.tile_pool(name="psH", bufs=2, space="PSUM"))
        psS = ctx.enter_context(tc.tile_pool(name="psS", bufs=1, space="PSUM"))

        # ---------------- constants ----------------
        # anything a f32r matmul consumes must be WRITTEN as f32r by a
        # compute engine (BIR verifier: "not rounded to FP32r"); gpsimd
        # can't emit f32r, so constants go fp32 -> ACT copy / DMA.
        identf = singles.tile([P, P], F32, tag="identf")
        make_identity(nc, identf)
        ident = singles.tile([P, P], F32, tag="ident")
        nc.scalar.copy(out=cast(ident), in_=identf)
        onesf = singles.tile([P, 2], F32, tag="onesf")
        nc.gpsimd.memset(onesf, 1.0)
        ones2 = singles.tile([P, 2], F32, tag="ones2")
        nc.scalar.copy(out=cast(ones2), in_=onesf)
        onesrowf = singles.tile([1, N_NODES], F32, tag="onesrowf")
        nc.gpsimd.memset(onesrowf, 1.0)
        # z operands: zl2 = [ones; s2], zr2 = [s1; ones].  ones rows: p0 via
        # ACT copy (f32r), p1 via DMA (engines can't address partition 1).
        zl2 = singles.tile([2, N_NODES], F32, tag="zl2")
        nc.scalar.copy(out=cast(zl2[0:1, :]), in_=onesrowf)
        zr2 = singles.tile([2, N_NODES], F32, tag="zr2")
        nc.sync.dma_start(out=cast(zr2[1:2, :]), in_=cast(onesrowf))

        a_flat = a_d.rearrange("f one -> (f one)")
        a1b = singles.tile([P, F], F32, tag="a1b")
        a2b = singles.tile([P, F], F32, tag="a2b")
        beta_sb = singles.tile([1, 1], F32, tag="beta_sb")
        w_all = singles.tile([P, NK * F], F32, tag="w_all")
        w12 = singles.tile([P, 2 * NK], F32, tag="w12")

        def load_weights():
            nc.sync.dma_start(out=a1b, in_=a_flat[0:F].partition_broadcast(P))
            nc.sync.dma_start(out=a2b, in_=a_flat[F : 2 * F].partition_broadcast(P))
            # beta lands in SBUF only to keep the input bound (value baked)
            nc.sync.dma_start(out=beta_sb, in_=beta_d[0:1].unsqueeze(0))
            nc.sync.dma_start(
                out=cast(w_all.rearrange("p (k f) -> p k f", k=NK)),
                in_=cast(w_d.rearrange("(k p) f -> p k f", p=P)),
            )
            # w12[:, 2k+j] = sum_f W_k[:, f] * a_j[f]  (DVE; idle during x fill)
            for k in range(NK):
                w12f = cpool.tile([P, 2], F32, tag="w12f")
                for j, ab in enumerate((a1b, a2b)):
                    prod = cpool.tile([P, F], F32, tag="prod")
                    nc.vector.tensor_tensor(
                        out=prod,
                        in0=w_all.bitcast(F32)[:, k * F : (k + 1) * F],
                        in1=ab,
                        op=AL.mult,
                    )
                    nc.vector.reduce_sum(
                        out=w12f[:, j : j + 1], in_=prod, axis=mybir.AxisListType.X
                    )
                nc.scalar.copy(out=cast(w12[:, 2 * k : 2 * k + 2]), in_=w12f)

        # ---------------- per-batch state ----------------
        xt_alls = {}
        h_sbs = {0: [], 1: []}
        uts = {0: [], 1: []}
        rcols = {}
        vpairs = {}
        x_tiles = {}

        def warmup(n):
            # dense dummy matmuls: hold the HAM activity window busy so the
            # PE clock ungates early; ap=128 -> ~210ns each while cold
            wp = psZ.tile([P, N_NODES], F32, tag="zps")
            for i in range(n):
                nc.tensor.matmul(
                    wp[:, (i % NN) * P : (i % NN + 1) * P],
                    lhsT=cast(ident),
                    rhs=cast(ident),
                    start=True,
                    stop=True,
                )

        def phase_A_dma(b):  # x as 4 [128,1024] pair tiles
            x_tiles[b] = []
            for m in range(NN // 2):
                x_t = xin.tile([P, 2 * F], F32, tag="x")
                nc.sync.dma_start(
                    out=cast(x_t.rearrange("p (t f) -> p t f", t=2)),
                    in_=cast(
                        x_d[b, m * 2 * P : (m + 1) * 2 * P, :].rearrange(
                            "(t p) f -> p t f", t=2
                        )
                    ),
                )
                x_tiles[b].append(x_t)

        def emit_A_tile(b, n, act_copy=False):
            x_pair = x_tiles[b][n // 2]
            base = (n % 2) * F
            xp = psZ.tile([P, N_NODES], F32, tag="zps")
            for k in range(NK):
                nc.tensor.transpose(
                    cast(xp[:, k * P : (k + 1) * P]),
                    cast(x_pair[:, base + k * P : base + (k + 1) * P]),
                    cast(ident),
                )
            dst = xt_alls[b].rearrange("p (k c) -> p k c", k=NK)[
                :, :, n * P : (n + 1) * P
            ]
            src = xp[:, 0:F].rearrange("p (k c) -> p k c", k=NK)
            if act_copy:
                nc.scalar.copy(out=cast(dst), in_=cast(src))
            else:
                nc.vector.tensor_copy(out=cast(dst), in_=cast(src))

        def phase_S(b):
            xt_all = xt_alls[b]
            s_ps = psS.tile([2, N_NODES], F32, tag="sps")
            for k in range(NK):
                for hh in range(2):
                    nc.tensor.matmul(
                        s_ps[:, hh * F : (hh + 1) * F],
                        lhsT=cast(w12[:, 2 * k : 2 * k + 2]),
                        rhs=cast(
                            xt_all[:, k * N_NODES + hh * F : k * N_NODES + (hh + 1) * F]
                        ),
                        start=(k == 0),
                        stop=(k == NK - 1),
                    )
            s_sb = spool.tile([2, N_NODES], F32, tag="s_sb")
            nc.vector.tensor_copy(out=s_sb, in_=s_ps)
            # zr2 p0 <- s1 (engine copy); zl2 p1 <- s2 (DMA: partition 1)
            nc.vector.tensor_copy(out=cast(zr2[0:1, :]), in_=cast(s_sb[0:1, :]))
            nc.sync.dma_start(out=cast(zl2[1:2, :]), in_=cast(s_sb[1:2, :]))

        def emit_B_tile(b, n, act_copy):
            xt_all = xt_alls[b]
            h_ps = psH.tile([P, F], F32, tag="hps")
            for k in range(NK):
                nc.tensor.matmul(
                    h_ps,
                    lhsT=cast(
                        xt_all[:, k * N_NODES + n * P : k * N_NODES + (n + 1) * P]
                    ),
                    rhs=cast(w_all[:, k * F : (k + 1) * F]),
                    start=(k == 0),
                    stop=(k == NK - 1),
                )
            ht = hpool.tile([P, F], F32, tag="h")
            if act_copy:
                nc.scalar.copy(out=cast(ht), in_=h_ps)
            else:
                nc.vector.tensor_copy(out=cast(ht), in_=h_ps)
            h_sbs[b].append(ht)

        def emit_C_tile(b, j, dve_lrelu):
            # uT[j][p, i] = exp(lrelu(s2[j*128+p] + s1[i]))
            z_ps = psZ.tile([P, N_NODES], F32, tag="zps")
            for hh in range(2):
                nc.tensor.matmul(
                    z_ps[:, hh * F : (hh + 1) * F],
                    lhsT=cast(zl2[:, j * P : (j + 1) * P]),
                    rhs=cast(zr2[:, hh * F : (hh + 1) * F]),
                    start=True,
                    stop=True,
                )
            lr = cpool.tile([P, N_NODES], F32, tag="lr")
            if dve_lrelu:
                # DVE lrelu: t = 0.2*z (PSUM->SB), lr = max(t, z); a single
                # STT can't read both operands from PSUM (NCC_IBVF027)
                t02 = cpool.tile([P, N_NODES], F32, tag="t02", bufs=1)
                nc.vector.tensor_scalar_mul(t02, z_ps, ALPHA)
                nc.vector.scalar_tensor_tensor(
                    out=lr, in0=t02, scalar=1.0, in1=z_ps, op0=AL.mult, op1=AL.max
                )
            else:
                nc.scalar.activation(out=lr, in_=z_ps, func=AF.Prelu, alpha=ALPHA)
            u = utp.tile([P, N_NODES], F32, tag="u")
            nc.scalar.activation(out=cast(u), in_=lr, func=AF.Exp)
            uts[b].append(u)

        def phase_R(b):  # rowsum -> reciprocal columns (DRAM roundtrip)
            ut = uts[b]
            rs_ps = psS.tile([2, N_NODES], F32, tag="sps")
            for j in range(NN):
                for hh in range(2):
                    nc.tensor.matmul(
                        rs_ps[:, hh * F : (hh + 1) * F],
                        lhsT=cast(ones2),
                        rhs=cast(ut[j][:, hh * F : (hh + 1) * F]),
                        start=(j == 0),
                        stop=(j == NN - 1),
                    )
            rrow = spool.tile([1, N_NODES], F32, tag="rrow")
            nc.vector.tensor_copy(out=rrow, in_=rs_ps[0:1, :])
            nc.sync.dma_start(out=r_d[b].unsqueeze(0), in_=rrow)
            rcraw = spool.tile([P, NN], F32, tag="rcraw", bufs=2)
            nc.sync.dma_start(out=rcraw, in_=r_d[b].rearrange("(n p) -> p n", p=P))
            rcol = spool.tile([P, NN], F32, tag="rcol", bufs=2)
            rcols[b] = rcol
            nc.vector.reciprocal(out=rcol, in_=rcraw)

        def emit_DE(b, n, act_variant):
            # p[n] = u @ h ; on odd n run the fused pair epilogue:
            # elu(v) = max(-relu(1 - exp(v)), v), v = p*rcol + beta*h
            ut, h_sb = uts[b], h_sbs[b]
            p_ps = psH.tile([P, F], F32, tag="hps")
            for j in range(NN):
                nc.tensor.matmul(
                    p_ps,
                    lhsT=cast(ut[j][:, n * P : (n + 1) * P]),
                    rhs=cast(h_sb[j]),
                    start=(j == 0),
                    stop=(j == NN - 1),
                )
            half = n % 2
            if half == 0:
                v_new = epool.tile([P, 2 * F], F32, tag="v", bufs=1)
                vpairs[b] = v_new
            v_pair = vpairs[b]
            hin = h_sb[n].bitcast(F32)
            if beta_val == 1.0:
                hb = hin
            else:
                hb = epool.tile([P, F], F32, tag="hb")
                nc.vector.tensor_scalar_mul(hb, hin, float(beta_val))
            nc.vector.scalar_tensor_tensor(
                out=v_pair[:, half * F : (half + 1) * F],
                in0=p_ps,
                scalar=rcols[b][:, n : n + 1],
                in1=hb,
                op0=AL.mult,
                op1=AL.add,
            )
            if half == 1:
                em = epool.tile([P, 2 * F], F32, tag="em", bufs=1)
                nc.scalar.activation(out=em, in_=v_pair, func=AF.Exp)
                tp = epool.tile([P, 2 * F], F32, tag="tp", bufs=1)
                o = epool.tile([P, 2 * F], F32, tag="o", bufs=1)
                if act_variant:
                    # tp = relu(1 - em); o = max(-tp, v)
                    nc.scalar.activation(
                        out=tp, in_=em, func=AF.Relu, bias=1.0, scale=-1.0
                    )
                    nc.vector.scalar_tensor_tensor(
                        out=o, in0=tp, scalar=-1.0, in1=v_pair, op0=AL.mult, op1=AL.max
                    )
                else:
                    # tp = min(em - 1, 0); o = max(tp, v)
                    nc.vector.tensor_scalar(
                        out=tp, in0=em, scalar1=-1.0, scalar2=0.0, op0=AL.add, op1=AL.min
                    )
                    nc.vector.tensor_tensor(out=o, in0=tp, in1=v_pair, op=AL.max)
                nc.sync.dma_start(
                    out=out_d[b, (n - 1) * P : (n + 1) * P, :].rearrange(
                        "(t p) f -> p t f", t=2
                    ),
                    in_=o.rearrange("p (t f) -> p t f", t=2),
                )

        # ---------------- emission schedule ----------------
        phase_A_dma(0)
        load_weights()
        warmup(8)
        xt0 = xtp.tile([P, NK * N_NODES], F32, tag="xt_all")
        xt_alls[0] = xt0
        for n in range(NN):
            emit_A_tile(0, n, act_copy=(n % 2 == 1))
            if n < 4:
                warmup(2)
        phase_S(0)
        phase_A_dma(1)
        for n in range(NN):
            emit_B_tile(0, n, act_copy=(n % 2 == 0))

        # --- superphase: C(0) interleaved with A(1) + B(1) ---
        xt1 = xtp.tile([P, NK * N_NODES], F32, tag="xt_all")
        xt_alls[1] = xt1
        for j in range(NN):
            emit_C_tile(0, j, dve_lrelu=(j in (3, 6)))
            emit_A_tile(1, j, act_copy=False)
            if j >= 3:
                emit_B_tile(1, j - 3, act_copy=False)
        emit_B_tile(1, 5, act_copy=False)
        phase_S(1)
        emit_B_tile(1, 6, act_copy=False)
        emit_B_tile(1, 7, act_copy=False)
        phase_R(0)

        # --- C(1) interleaved with DE(0) ---
        # epilogue variants per pair: b0 [A,D,D,D], b1 [A,D,A,D]
        EPI = {0: [False, False, False, False], 1: [True, False, True, True]}
        for j in range(3):
            emit_C_tile(1, j, dve_lrelu=False)
        for n in range(5):
            emit_DE(0, n, act_variant=EPI[0][n // 2])
            emit_C_tile(1, n + 3, dve_lrelu=((n + 3) in (3, 6)))
        emit_DE(0, 5, act_variant=EPI[0][2])
        emit_DE(0, 6, act_variant=EPI[0][3])
        phase_R(1)
        emit_DE(0, 7, act_variant=EPI[0][3])
        for n in range(NN):
            emit_DE(1, n, act_variant=EPI[1][n // 2])

    nc.finalize()
    return nc


_NC_CACHE = {}


def _get_nc(mm_fp32: bool, beta_val: float) -> bass.Bass:
    key = (bool(mm_fp32), float(beta_val))
    if key not in _NC_CACHE:
        _NC_CACHE[key] = build_nc(mm_fp32=key[0], beta_val=key[1])
    return _NC_CACHE[key]


def kernel(x, W, a, beta, _trace=False, _mm_fp32=False):
    x = np.ascontiguousarray(x, dtype=np.float32)
    W = np.ascontiguousarray(W, dtype=np.float32)
    a = np.ascontiguousarray(a, dtype=np.float32)
    beta = np.ascontiguousarray(beta, dtype=np.float32)

    nc = _get_nc(_mm_fp32, float(beta.reshape(-1)[0]))
    in_maps = [
        {
            "x": x[c * B_PER_CORE : (c + 1) * B_PER_CORE],
            "W": W,
            "a": a,
            "beta": beta,
        }
        for c in range(N_CORES)
    ]
    res = run_bass_kernel_spmd(nc, in_maps, core_ids=list(range(N_CORES)), trace=_trace)
    out = np.concatenate([r["out"] for r in res.results], axis=0)
    if _trace:
        kernel.last_exec_time_ns = res.exec_time_ns
        kernel.last_results = res
    return out


if __name__ == "__main__":
    rng = np.random.default_rng(0)
    x = rng.standard_normal((B_TOTAL, N_NODES, F), dtype=np.float32)
    W = rng.standard_normal((F, F), dtype=np.float32) * 0.05
    a = rng.standard_normal((2 * F, 1), dtype=np.float32) * 0.05
    beta = np.ones((1,), dtype=np.float32)
    out = kernel(x, W, a, beta)
    print("out", out.shape, out.dtype)
